# revision 5
# baseline (speedup 1.0000x reference)
"""Trainium2 Bass kernel for attention-score softmax.

Computes, for input_sec [B=8, S=8192, D=1024], state [B, D], w [D], b [1]:
    energy[b, s] = dot(tanh(input_sec[b, s, :] + state[b, :]), w) + b
    out[b, :]    = softmax(energy[b, :], axis=-1)

Sharding: data-parallel over batch, one batch element per NeuronCore (8 cores).
Per-core dataflow (on transposed input xT [D, S], prepared host-side):
  - DMA xT d-block tiles [128, S_GROUP]
  - ScalarE: t = tanh(x + state[d])   (state is a per-partition bias column)
  - TensorE: energy = w . t, accumulated over the 8 d-blocks into a single
    PSUM tile [16, 512]; sequence chunk j lands on PSUM partition j via
    block-diagonal weight columns (lhsT column j = w, other columns zero).
  - ScalarE: p = exp(energy) with fused per-partition row sums (accum_out).
    (softmax max-subtraction is skipped: |energy| <= ||w||_1 ~ 26, exp is
     safely inside fp32 range, and softmax is shift-invariant so the bias b
     never affects the output at all.)
  - TensorE: ones-matmul reduces the 16 row sums and broadcasts the total
    back to 16 partitions; VectorE reciprocal + scale; DMA out.
"""

import os
from contextlib import ExitStack

import numpy as np

import concourse.bacc as bacc
import concourse.tile as tile
from concourse import mybir
from concourse.bass_utils import run_bass_kernel_spmd

B, S, D = 8, 8192, 1024
NB_D = D // 128          # 8 d-blocks
N_CHUNK = S // 512       # 16 sequence chunks of 512
S_GROUP = 2048           # sequence tile width for DMA/tanh pipelining
NG = S // S_GROUP
CH_PER_G = S_GROUP // 512

# input dtype on device: "fp16" halves DMA traffic (memory-bound kernel);
# "fp32" is the conservative fallback.
X_DTYPE = os.environ.get("ATTN_KERNEL_XDTYPE", "fp16")

_compiled = {}
last_result = None  # BassKernelResults of the most recent run (for test harness)


def _build(x_dtype: str):
    xdt = mybir.dt.float16 if x_dtype == "fp16" else mybir.dt.float32
    f32 = mybir.dt.float32

    nc = bacc.Bacc()
    xT = nc.declare_dram_parameter("xT", [D, S], xdt, isOutput=False)
    state_cols = nc.declare_dram_parameter("state_cols", [128, NB_D], f32,
                                           isOutput=False)
    w_blk = nc.declare_dram_parameter("w_blk", [NB_D, 128, 16 * 16], xdt,
                                      isOutput=False)
    out_ext = nc.declare_dram_parameter("out", [S], f32, isOutput=True)

    with tile.TileContext(nc) as tc, ExitStack() as ctx:
        consts = ctx.enter_context(tc.tile_pool(name="consts", bufs=1))
        xpool = ctx.enter_context(tc.tile_pool(name="x", bufs=2 * NB_D))
        tpool = ctx.enter_context(tc.tile_pool(name="t", bufs=2 * NB_D))
        tailp = ctx.enter_context(tc.tile_pool(name="tail", bufs=1))
        psum = ctx.enter_context(tc.tile_pool(name="psum", bufs=2, space="PSUM"))

        state_sb = consts.tile([128, NB_D], f32)
        nc.gpsimd.dma_start(out=state_sb, in_=state_cols[:])
        w_sb = consts.tile([128, NB_D, 256], xdt)
        nc.gpsimd.dma_start(out=w_sb, in_=w_blk[:].rearrange("i p c -> p i c"))
        ones_sb = consts.tile([128, 16], f32)
        nc.vector.memset(ones_sb, 1.0)
        sums_sb = consts.tile([128, 1], f32)
        nc.vector.memset(sums_sb, 0.0)

        energy_ps = psum.tile([16, 512], f32)

        n_mm = 0
        for g in range(NG):
            xt = []
            for i in range(NB_D):
                x_t = xpool.tile([128, S_GROUP], xdt, tag="x")
                nc.sync.dma_start(
                    out=x_t,
                    in_=xT[:][128 * i:128 * (i + 1),
                              S_GROUP * g:S_GROUP * (g + 1)],
                )
                t_t = tpool.tile([128, S_GROUP], xdt, tag="t")
                nc.scalar.activation(
                    out=t_t, in_=x_t,
                    func=mybir.ActivationFunctionType.Tanh,
                    bias=state_sb[:, i:i + 1], scale=1.0,
                )
                xt.append(t_t)
            for c in range(CH_PER_G):
                j = CH_PER_G * g + c
                for i in range(NB_D):
                    n_mm += 1
                    nc.tensor.matmul(
                        energy_ps[:],
                        lhsT=w_sb[:, i, 16 * j:16 * (j + 1)],
                        rhs=xt[i][:, 512 * c:512 * (c + 1)],
                        start=(n_mm == 1),
                        stop=(n_mm == NG * CH_PER_G * NB_D),
                    )

        # softmax tail
        p_sb = tailp.tile([16, 512], f32)
        nc.scalar.activation(
            out=p_sb, in_=energy_ps[:],
            func=mybir.ActivationFunctionType.Exp,
            bias=0.0, scale=1.0,
            accum_out=sums_sb[0:16, :],
        )
        sum_ps = psum.tile([16, 1], f32)
        nc.tensor.matmul(sum_ps[:], lhsT=ones_sb, rhs=sums_sb,
                         start=True, stop=True)
        inv_sb = tailp.tile([16, 1], f32)
        nc.vector.reciprocal(out=inv_sb, in_=sum_ps[:])
        out_sb = tailp.tile([16, 512], f32)
        nc.vector.tensor_scalar_mul(out=out_sb, in0=p_sb, scalar1=inv_sb)
        nc.gpsimd.dma_start(
            out=out_ext[:].rearrange("(p f) -> p f", p=16), in_=out_sb,
        )

    nc.finalize()
    return nc


def _get_nc(x_dtype: str):
    if x_dtype not in _compiled:
        _compiled[x_dtype] = _build(x_dtype)
    return _compiled[x_dtype]


def kernel(input_sec, state, w, b, **_unused):
    x_dtype = X_DTYPE
    np_xdt = np.float16 if x_dtype == "fp16" else np.float32
    nc = _get_nc(x_dtype)

    # host-side layout prep
    xT_all = np.ascontiguousarray(
        input_sec.transpose(0, 2, 1)).astype(np_xdt)          # [B, D, S]
    state_cols_all = np.ascontiguousarray(
        np.asarray(state, np.float32).reshape(B, NB_D, 128).transpose(0, 2, 1)
    )                                                          # [B, 128, NB_D]
    w_grid = np.asarray(w, np.float32).reshape(NB_D, 128)
    w_blk = np.zeros((NB_D, 128, 16, 16), np.float32)
    for j in range(16):
        w_blk[:, :, j, j] = w_grid
    w_blk = w_blk.reshape(NB_D, 128, 256).astype(np_xdt)

    in_maps = [
        {
            "xT": np.ascontiguousarray(xT_all[c]),
            "state_cols": state_cols_all[c],
            "w_blk": w_blk,
        }
        for c in range(B)
    ]
    trace = bool(int(os.environ.get("ATTN_KERNEL_TRACE", "0")))
    res = run_bass_kernel_spmd(nc, in_maps, core_ids=list(range(B)),
                               trace=trace)
    global last_result
    last_result = res
    out = np.stack([res.results[c]["out"] for c in range(B)], axis=0)
    return out.astype(np.float32)
